# revision 39
# baseline (speedup 1.0000x reference)
"""Trainium2 Bass kernel for nn_Compressor (sparse_attention block compressor).

Math (reference):
  proj = x @ [W_kv; W_gate]^T            # [b*s, 2048]
  kv   = proj[:, :1024] + ape[s%4]       # blockwise (RATIO=4) abs-pos bias
  sc   = proj[:, 1024:]
  window(blk) = {prev blk rows, ch 0:512} + {cur blk rows, ch 512:1024}
  pooled[blk, c] = softmax-gated channelwise pool over the 8-entry window
  out = (RMSNorm(pooled) -> rope on ch 448:512) @ H  (512x512 Hadamard)

Distribution: 8 cores, data-parallel over (batch, seq-half). Each core owns
2048 seq rows = 512 blocks; the 1-block halo is handled by shifting the
matmul rhs window by 4 rows (xs input carries 16 halo rows).

Schedule notes:
  * PE does 2048 projection matmuls (N=512, ~213ns each) plus small extras
    (ape-bias K=4 matmuls, bf16 Hadamard/RMS-stats); everything else is
    scheduled around keeping its issue rate at 1/213ns:
    - ape bias folded into the psum group as a K=4 matmul on a one-hot
      phase-indicator rhs, so kv psums need no drain: the pooling multiply
      reads them in place (score chunks run first in each group so kv psum
      banks get a long reuse window)
    - (mch0, j0) runs d-outer so the PE ramps at HBM delivery rate; weight
      DMAs stream from the ACT queue, big consts deferred past the head
    - per-j Hadamard/RMS-stats matmuls deferred one proj group so their
      waits resolve before they reach the PE queue head; whole chunk tails
      deferred into the next chunk's first proj group
    - a short dummy-matmul warmup raises the PE p-state during head DMAs
  * Pooling on split engines: exp windows land in one [128,b,8] bf16 tile
    (single reduce for the softmax denominator), products in one bf16 tile
    (single reduce for the numerator); final divide via DVE reciprocal,
    normalize multiply + squares on GpSimd. The rope pair-swap is a DVE
    stream_shuffle on partitions 64:128 (cos=1/sin=0 below), no PE/PSUM.
  * RMSNorm channel reduction via lhsT=squared-pooled, rhs=ones -> var lands
    as a [128,1] psum column (no transpose); rsqrt via DVE-only Newton
    iterations so ACT keeps a single exp/copy table the whole kernel.
  * Softmax without max-subtraction (scores ~N(0,1.3); fp32 exp cannot
    overflow; block-0 masking is a 0/1 multiply on exp, per-core mask).
"""

import os
import numpy as np
import ml_dtypes

import concourse.bass as bass
import concourse.bacc as bacc
import concourse.mybir as mybir
from concourse.alu_op_type import AluOpType
from concourse.tile import TileContext
from concourse.bass_utils import run_bass_kernel_spmd

BF16 = ml_dtypes.bfloat16
F32 = mybir.dt.float32
BF = mybir.dt.bfloat16

N_CORES = 8
DIM = 4096
OCH = 2048          # kv 1024 + gate 1024
ROWS = 2048         # own rows per core
XS_ROWS = 2064      # 16 halo/pad rows + 2048
MCH = 4             # m-chunks per core
MROWS = 512         # rows per m-chunk
NBLK = 128          # blocks per m-chunk
DCH = 32            # d chunks of 128
OCHK = 16           # o chunks of 128
# o-chunks 0..3 kv-first(prev), 4..7 kv-second(cur), 8..11 sc-first, 12..15 sc-second
FIRST_HALF = (0, 1, 2, 3, 8, 9, 10, 11)

_CACHE = {}


def _build():
    nc = bacc.Bacc("TRN2", target_bir_lowering=False, debug=False,
                   num_devices=N_CORES)
    xs = nc.dram_tensor("xs", [DIM, XS_ROWS], BF, kind="ExternalInput")
    wp = nc.dram_tensor("wp", [OCHK, 128, DCH, 128], BF, kind="ExternalInput")
    apekv_d = nc.dram_tensor("ape_kv", [4, 8, 128], BF, kind="ExternalInput")
    ind_d = nc.dram_tensor("ind", [4, 528], BF, kind="ExternalInput")
    cos_d = nc.dram_tensor("cos_t", [128, 512], F32, kind="ExternalInput")
    sin_d = nc.dram_tensor("sin_t", [128, 512], F32, kind="ExternalInput")
    h_d = nc.dram_tensor("hmat", [128, 4, 512], BF, kind="ExternalInput")
    zmask_d = nc.dram_tensor("zmask", [128, 1], F32, kind="ExternalInput")
    out_d = nc.dram_tensor("out", [4 * NBLK, 512], F32, kind="ExternalOutput")

    X = mybir.AxisListType.X

    def g4(tile_ap):
        return tile_ap.rearrange("p (b r) -> p b r", r=4)

    with TileContext(nc) as tc:
        with (
            tc.tile_pool(name="const", bufs=1) as constp,
            tc.tile_pool(name="xt", bufs=2) as xtp,
            tc.tile_pool(name="wt", bufs=7) as wtp,
            tc.tile_pool(name="sb", bufs=2) as sbp,
            tc.tile_pool(name="pl", bufs=2) as plp,
            tc.tile_pool(name="sm", bufs=2) as smp,
            tc.tile_pool(name="osb", bufs=2) as outp,
            tc.tile_pool(name="proj", bufs=6, space="PSUM") as projp,
            tc.tile_pool(name="had", bufs=1, space="PSUM") as hadp,
            tc.tile_pool(name="var", bufs=1, space="PSUM") as varp,
        ):
            # ---- weight prefetch for the first j-group goes out first ----
            w_tiles = {}
            for oc in (8, 12, 0, 4):
                w = wtp.tile([128, DCH, 128], BF, tag="w")
                w_tiles[oc] = w
            for k in range(0, DCH, 8):
                for oc in (8, 12, 0, 4):
                    nc.scalar.dma_start(out=w_tiles[oc][:, k:k + 8, :],
                                        in_=wp[oc][:, k:k + 8, :])

            # ---- constants: ape/ind/zmask first (needed by first groups) ----
            apekv_sb = constp.tile([4, 8, 128], BF, tag="apekv")
            nc.gpsimd.dma_start(out=apekv_sb[:], in_=apekv_d[:, :, :])
            ind_sb = constp.tile([4, 528], BF, tag="ind")
            nc.gpsimd.dma_start(out=ind_sb[:], in_=ind_d[:, :])
            zmask_sb = constp.tile([128, 1], F32, tag="zmask")
            nc.gpsimd.dma_start(out=zmask_sb[:], in_=zmask_d[:, :])
            cos_sb = constp.tile([128, 512], F32, tag="cos")
            sin_sb = constp.tile([128, 512], F32, tag="sin")
            h_sb = constp.tile([128, 4, 512], BF, tag="h")
            ones_sb = constp.tile([128, 1], BF, tag="ones")
            nc.vector.memset(ones_sb[:], 1.0)

            # PE p-state warmup: a few dummy matmuls while the first
            # weight/x DMAs are in flight, so real matmuls start at 2.4GHz
            warm_rhs = constp.tile([128, 512], BF, tag="warm")
            nc.gpsimd.memset(warm_rhs[:], 0.0)
            warm_ps = hadp.tile([128, 512], F32, tag="had", name="warm_ps")
            for wi in range(8):
                nc.tensor.matmul(warm_ps[:], lhsT=warm_rhs[:, 0:128],
                                 rhs=warm_rhs[:],
                                 start=(wi % 4 == 0), stop=(wi % 4 == 3))


            # deferred per-chunk state
            prev = None     # state dict of the previous m-chunk, for tail MMs
            for mch in range(MCH):
                r0 = MROWS * mch
                # ---- x^T tile: [128(d), 32 dchunk, 528 m] bf16 ----
                xt = xtp.tile([128, DCH, 528], BF, tag="xt")
                nbatch = 2 if mch == 0 else 4
                for c in range(0, DCH, nbatch):
                    nc.sync.dma_start(
                        out=xt[:, c:c + nbatch, :],
                        in_=xs[128 * c:128 * (c + nbatch), r0:r0 + 528]
                            .rearrange("(c p) m -> p c m", p=128),
                    )

                group = {}
                pooled = plp.tile([128, 4, NBLK], BF, tag="pooled")
                sq = plp.tile([128, 4, NBLK], BF, tag="sq")
                state = dict(pooled=pooled, sq=sq, mch=mch, emitted=0)

                def emit_stats_had(st, j):
                    """PE-side per-j tail: RMS-stats MM + Hadamard MM for
                    group j of chunk st (deferred until deps are ready)."""
                    p, s = st["pooled"], st["sq"]
                    nc.tensor.matmul(
                        st["var"][:], lhsT=s[:, j, :], rhs=ones_sb[:, 0:1],
                        start=(j == 0), stop=(j == 3), skip_group_check=True)
                    nc.tensor.matmul(
                        st["had"][:], lhsT=p[:, j, :], rhs=h_sb[:, j, :],
                        start=(j == 0), stop=(j == 3), skip_group_check=True)

                def emit_chunk_tail(st):
                    """rope + j=3 stats/had + scale + copy-out for chunk st."""
                    p = st["pooled"]
                    m = st["mch"]
                    # rope on group 3: cos=1/sin=0 on partitions <64, so
                    # only 64:128 need work; the pair swap is a DVE
                    # stream_shuffle (quadrant mask i^1), no PE/PSUM involved
                    cslice = cos_sb[64:128, m * NBLK:(m + 1) * NBLK]
                    sslice = sin_sb[64:128, m * NBLK:(m + 1) * NBLK]
                    sw = smp.tile([128, NBLK], BF, tag="sw")
                    nc.vector.stream_shuffle(sw[64:128, :], p[64:128, 3, :],
                                             [i ^ 1 for i in range(32)])
                    tmpc = smp.tile([128, NBLK], F32, tag="tmpc")
                    nc.vector.tensor_mul(tmpc[64:128, :], p[64:128, 3, :],
                                         cslice)
                    tmps = smp.tile([128, NBLK], F32, tag="tmps")
                    nc.vector.tensor_mul(tmps[64:128, :], sw[64:128, :],
                                         sslice)
                    nc.vector.tensor_add(p[64:128, 3, :], tmpc[64:128, :],
                                         tmps[64:128, :])
                    # j=3 squared stats on gpsimd (pre-rope value already in sq)
                    emit_stats_had(st, 3)
                    # scale = rsqrt(var/512 + eps) via DVE-only Newton
                    # iterations ([128,1] ops), so ACT keeps a single
                    # exp/copy table the whole kernel. Linear seed fitted
                    # for v in [0.25, 1.2]; two iterations -> <0.2% error.
                    v0 = smp.tile([128, 1], F32, tag="v0")
                    nc.vector.tensor_scalar(v0[:], st["var"][:],
                                            1.0 / 512.0, 1e-6,
                                            AluOpType.mult, AluOpType.add)
                    rsq = smp.tile([128, 1], F32, tag="rsq")
                    nc.vector.tensor_scalar(rsq[:], v0[:],
                                            -1.0424, 1.9774,
                                            AluOpType.mult, AluOpType.add)
                    tn = smp.tile([128, 1], F32, tag="tn")
                    for _ in range(2):
                        nc.vector.tensor_mul(tn[:], rsq[:], rsq[:])
                        nc.vector.tensor_mul(tn[:], tn[:], v0[:])
                        nc.vector.tensor_scalar(tn[:], tn[:], -0.5, 1.5,
                                                AluOpType.mult, AluOpType.add)
                        nc.vector.tensor_mul(rsq[:], rsq[:], tn[:])
                    out_sb = outp.tile([128, 512], F32, tag="out")
                    nc.scalar.activation(out_sb[:], st["had"][:],
                                         mybir.ActivationFunctionType.Copy,
                                         scale=rsq[:, 0:1])
                    nc.sync.dma_start(
                        out=out_d[m * NBLK:(m + 1) * NBLK, :], in_=out_sb[:])

                for j in range(4):
                    slots = (j + 8, j + 12, j, j + 4)
                    if mch == 0 and j == 0:
                        # d-outer for the very first group: each xt d-chunk is
                        # consumed by all 4 ocs right after its DMA lands, so
                        # the PE ramps at the HBM delivery rate instead of
                        # stalling a full xt pass per oc.
                        pss = []
                        for t, oc in enumerate(slots):
                            pss.append(projp.tile([128, MROWS], F32,
                                                  tag="proj", name=f"ps0_{t}"))
                        for d in range(DCH):
                            for t, oc in enumerate(slots):
                                off = 12 if oc in FIRST_HALF else 16
                                nc.tensor.matmul(
                                    pss[t][:],
                                    lhsT=w_tiles[oc][:, d, :],
                                    rhs=xt[:, d, off:off + MROWS],
                                    start=(d == 0),
                                    stop=(d == DCH - 1) and oc >= 8,
                                )
                        for t, oc in enumerate(slots):
                            wn = wtp.tile([128, DCH, 128], BF, tag="w")
                            nc.scalar.dma_start(out=wn[:], in_=wp[oc + 1])
                            w_tiles[oc + 1] = wn
                        for t, oc in enumerate(slots):
                            ps = pss[t]
                            off = 12 if oc in FIRST_HALF else 16
                            if oc < 8:
                                nc.tensor.matmul(
                                    ps[:], lhsT=apekv_sb[:, oc, :],
                                    rhs=ind_sb[:, off:off + MROWS],
                                    start=False, stop=True)
                                group[f"kvps{t - 2}"] = ps
                            else:
                                if t == 0:
                                    ec = sbp.tile([128, NBLK, 8], BF, tag="ec")
                                    group["ec"] = ec
                                ec = group["ec"]
                                nc.scalar.activation(
                                    ec[:, :, 4 * t:4 * t + 4], g4(ps[:]),
                                    mybir.ActivationFunctionType.Exp)
                                if oc < 12:
                                    nc.gpsimd.tensor_scalar_mul(
                                        ec[:, 0, 0:4], ec[:, 0, 0:4],
                                        zmask_sb[:, 0:1])
                    for t, oc in enumerate(slots):
                        if mch == 0 and j == 0:
                            break
                        # prefetch weights for the same t-slot of next group
                        if j < 3 or mch < MCH - 1:
                            noc = oc + 1 if j < 3 else oc - 3
                            wn = wtp.tile([128, DCH, 128], BF, tag="w")
                            eng = nc.scalar if t % 2 == 0 else nc.sync
                            eng.dma_start(out=wn[:], in_=wp[noc])
                        else:
                            wn = None
                        w = w_tiles[oc]
                        ps = projp.tile([128, MROWS], F32, tag="proj")
                        off = 12 if oc in FIRST_HALF else 16
                        is_kv = oc < 8
                        for d in range(DCH):
                            nc.tensor.matmul(
                                ps[:],
                                lhsT=w[:, d, :],
                                rhs=xt[:, d, off:off + MROWS],
                                start=(d == 0),
                                stop=(d == DCH - 1) and not is_kv,
                            )
                        if is_kv:
                            # +ape bias: K=4 matmul with one-hot phase rhs
                            nc.tensor.matmul(
                                ps[:],
                                lhsT=apekv_sb[:, oc, :],
                                rhs=ind_sb[:, off:off + MROWS],
                                start=False, stop=True,
                            )
                            group[f"kvps{t - 2}"] = ps
                        w_tiles[oc + 1 if j < 3 else oc - 3] = wn
                        # deferred PE tail work now that a proj group separates
                        # it from its producers
                        if t == 0 and j == 0 and prev is not None:
                            emit_chunk_tail(prev)
                            prev = None
                        if t == 2 and j > 0 and state["emitted"] < j:
                            emit_stats_had(state, j - 1)
                            state["emitted"] = j
                        if j == 1 and t == 0 and mch == 0:
                            # hmat feeds the first deferred had-MM (j1,t2)
                            nc.scalar.dma_start(out=h_sb[:], in_=h_d[:, :, :])
                        if j == 2 and t < 2 and mch == 0:
                            # rope constants: first consumed at mch1 j0;
                            # spread across j2's slots so no single weight
                            # prefetch is pushed far back
                            cd = ((cos_sb, cos_d[:, :]), (sin_sb, sin_d[:, :]))
                            nc.scalar.dma_start(out=cd[t][0][:], in_=cd[t][1])
                        if not is_kv:
                            # score chunk: e = exp(psum) into its half of the
                            # combined 8-slot window tile (bf16)
                            if t == 0:
                                ec = sbp.tile([128, NBLK, 8], BF, tag="ec")
                                group["ec"] = ec
                            ec = group["ec"]
                            nc.scalar.activation(
                                ec[:, :, 4 * t:4 * t + 4], g4(ps[:]),
                                mybir.ActivationFunctionType.Exp)
                            if mch == 0 and oc < 12:
                                nc.gpsimd.tensor_scalar_mul(
                                    ec[:, 0, 0:4], ec[:, 0, 0:4],
                                    zmask_sb[:, 0:1])

                    if j == 0:
                        state["var"] = varp.tile([128, 1], F32, tag="var", name="var_ps")
                        state["had"] = hadp.tile([128, 512], F32, tag="had", name="had_ps")

                    ec = group["ec"]
                    kvps1, kvps2 = group["kvps0"], group["kvps1"]
                    ssum = smp.tile([128, NBLK], F32, tag="ssum")
                    nc.vector.reduce_sum(ssum[:], ec[:], axis=X)
                    rinv = smp.tile([128, NBLK], F32, tag="rinv")
                    nc.vector.reciprocal(rinv[:], ssum[:])

                    # weighted-value chain on DVE, reading kv psum in place
                    pm = sbp.tile([128, NBLK, 8], BF, tag="pm")
                    nc.vector.tensor_mul(pm[:, :, 0:4], ec[:, :, 0:4],
                                         g4(kvps1[:]))
                    nc.vector.tensor_mul(pm[:, :, 4:8], ec[:, :, 4:8],
                                         g4(kvps2[:]))
                    qsum = smp.tile([128, NBLK], F32, tag="qsum")
                    nc.vector.reduce_sum(qsum[:], pm[:], axis=X)
                    nc.gpsimd.tensor_mul(pooled[:, j, :], qsum[:], rinv[:])
                    # squared pooled for RMS stats (pre-rope), on gpsimd
                    nc.gpsimd.tensor_mul(sq[:, j, :], pooled[:, j, :],
                                         pooled[:, j, :])

                prev = state
            emit_chunk_tail(prev)
    nc.compile()
    return nc


def _prep_shared(W_kv, W_gate, ape, norm_w, H):
    W = np.concatenate([W_kv, W_gate], axis=0).astype(np.float32)  # [2048, 4096]
    Wb = W.astype(BF16)
    wp = np.ascontiguousarray(
        Wb.T.reshape(DCH, 128, OCHK, 128).transpose(2, 1, 0, 3))  # [16,128,32,128]
    # ape bias as a K=4 matmul operand: ape_kv[r, oc, m] = ape[r, 128*oc+m]
    ape_kv = np.ascontiguousarray(
        ape.astype(np.float32)[:, :1024].reshape(4, 8, 128)).astype(BF16)
    ind = np.zeros((4, 528), np.float32)
    mm = np.arange(528)
    ind[(mm - 16) % 4, mm] = 1.0
    ind = ind.astype(BF16)
    hm = np.ascontiguousarray(
        (norm_w.astype(np.float32)[:, None] * H.astype(np.float32))
        .reshape(4, 128, 512).transpose(1, 0, 2)).astype(BF16)
    return wp, ape_kv, ind, hm


def _hadamard(n):
    h = np.array([[1.0]], dtype=np.float32)
    while h.shape[0] < n:
        h = np.block([[h, h], [h, -h]])
    return (h / np.sqrt(n)).astype(np.float32)


def _make_in_maps(x, W_kv, W_gate, ape, norm_w, freqs_cis):
    b, s, _ = x.shape
    H = _hadamard(512)
    wp, ape_kv, ind, hm = _prep_shared(W_kv, W_gate, ape, norm_w, H)

    # truncate-to-bf16 (hi-16 planes of the f32 words) and transpose once
    xh = x.reshape(b * s, DIM).view(BF16)[:, 1::2]
    xT = np.ascontiguousarray(xh.T)  # [4096, 16384]
    fr = freqs_cis[:, :, 0]  # [nb, 32]
    fi = freqs_cis[:, :, 1]

    in_maps = []
    for c in range(N_CORES):
        batch, half = c // 2, c % 2
        R0 = batch * s + half * ROWS
        xs = np.zeros((DIM, XS_ROWS), BF16)
        xs[:, 16:] = xT[:, R0:R0 + ROWS]
        if half == 1:
            xs[:, :16] = xT[:, R0 - 16:R0]

        g0 = half * 512
        bi = np.arange(g0, g0 + 512)
        cos_t = np.zeros((128, 512), np.float32)
        cos_t[:64] = 1.0
        cos_t[64:] = np.repeat(fr[bi].T, 2, axis=0)
        sin_t = np.zeros((128, 512), np.float32)
        st = np.repeat(fi[bi].T, 2, axis=0)
        st[0::2] *= -1.0
        sin_t[64:] = st

        zmask = np.full((128, 1), 0.0 if half == 0 else 1.0, np.float32)
        in_maps.append({
            "xs": xs, "wp": wp, "ape_kv": ape_kv, "ind": ind,
            "cos_t": cos_t, "sin_t": sin_t,
            "hmat": hm, "zmask": zmask,
        })
    return in_maps


def kernel(x, W_kv, W_gate, ape, norm_w, freqs_cis, start_pos=0):
    x = np.asarray(x, dtype=np.float32)
    W_kv = np.asarray(W_kv, dtype=np.float32)
    W_gate = np.asarray(W_gate, dtype=np.float32)
    ape = np.asarray(ape, dtype=np.float32)
    norm_w = np.asarray(norm_w, dtype=np.float32)
    freqs_cis = np.asarray(freqs_cis, dtype=np.float32)

    b, s, _ = x.shape
    nb = s // 4
    assert (b, s) == (4, 4096), (b, s)

    if "nc" not in _CACHE:
        _CACHE["nc"] = _build()
    nc = _CACHE["nc"]

    in_maps = _make_in_maps(x, W_kv, W_gate, ape, norm_w, freqs_cis)

    trace = os.environ.get("KERNEL_TRACE", "") not in ("", "0")
    res = run_bass_kernel_spmd(nc, in_maps, core_ids=list(range(N_CORES)),
                               trace=trace)
    kernel.last_results = res
    out = np.concatenate([res.results[c]["out"] for c in range(N_CORES)], axis=0)
    return np.ascontiguousarray(out.reshape(b, nb, 512))


# revision 40
# speedup vs baseline: 1.0012x; 1.0012x over previous
"""Trainium2 Bass kernel for nn_Compressor (sparse_attention block compressor).

Math (reference):
  proj = x @ [W_kv; W_gate]^T            # [b*s, 2048]
  kv   = proj[:, :1024] + ape[s%4]       # blockwise (RATIO=4) abs-pos bias
  sc   = proj[:, 1024:]
  window(blk) = {prev blk rows, ch 0:512} + {cur blk rows, ch 512:1024}
  pooled[blk, c] = softmax-gated channelwise pool over the 8-entry window
  out = (RMSNorm(pooled) -> rope on ch 448:512) @ H  (512x512 Hadamard)

Distribution: 8 cores, data-parallel over (batch, seq-half). Each core owns
2048 seq rows = 512 blocks; the 1-block halo is handled by shifting the
matmul rhs window by 4 rows (xs input carries 16 halo rows).

Schedule notes:
  * PE does 2048 projection matmuls (N=512, ~213ns each) plus small extras
    (ape-bias K=4 matmuls, bf16 Hadamard/RMS-stats); everything else is
    scheduled around keeping its issue rate at 1/213ns:
    - ape bias folded into the psum group as a K=4 matmul on a one-hot
      phase-indicator rhs, so kv psums need no drain: the pooling multiply
      reads them in place (score chunks run first in each group so kv psum
      banks get a long reuse window)
    - (mch0, j0) runs d-outer so the PE ramps at HBM delivery rate; weight
      DMAs stream from the ACT queue, big consts deferred past the head
    - per-j Hadamard/RMS-stats matmuls deferred one proj group so their
      waits resolve before they reach the PE queue head; whole chunk tails
      deferred into the next chunk's first proj group
    - a short dummy-matmul warmup raises the PE p-state during head DMAs
  * Pooling on split engines: exp windows land in one [128,b,8] bf16 tile
    (single reduce for the softmax denominator), products in one bf16 tile
    (single reduce for the numerator); final divide via DVE reciprocal,
    normalize multiply + squares on GpSimd. The rope pair-swap is a DVE
    stream_shuffle on partitions 64:128 (cos=1/sin=0 below), no PE/PSUM.
  * RMSNorm channel reduction via lhsT=squared-pooled, rhs=ones -> var lands
    as a [128,1] psum column (no transpose); rsqrt via DVE-only Newton
    iterations so ACT keeps a single exp/copy table the whole kernel.
  * Softmax without max-subtraction (scores ~N(0,1.3); fp32 exp cannot
    overflow; block-0 masking is a 0/1 multiply on exp, per-core mask).
"""

import os
import numpy as np
import ml_dtypes

import concourse.bass as bass
import concourse.bacc as bacc
import concourse.mybir as mybir
from concourse.alu_op_type import AluOpType
from concourse.tile import TileContext
from concourse.bass_utils import run_bass_kernel_spmd

BF16 = ml_dtypes.bfloat16
F32 = mybir.dt.float32
BF = mybir.dt.bfloat16

N_CORES = 8
DIM = 4096
OCH = 2048          # kv 1024 + gate 1024
ROWS = 2048         # own rows per core
XS_ROWS = 2064      # 16 halo/pad rows + 2048
MCH = 4             # m-chunks per core
MROWS = 512         # rows per m-chunk
NBLK = 128          # blocks per m-chunk
DCH = 32            # d chunks of 128
OCHK = 16           # o chunks of 128
# o-chunks 0..3 kv-first(prev), 4..7 kv-second(cur), 8..11 sc-first, 12..15 sc-second
FIRST_HALF = (0, 1, 2, 3, 8, 9, 10, 11)

_CACHE = {}


def _build():
    nc = bacc.Bacc("TRN2", target_bir_lowering=False, debug=False,
                   num_devices=N_CORES)
    xs = nc.dram_tensor("xs", [DIM, XS_ROWS], BF, kind="ExternalInput")
    wp = nc.dram_tensor("wp", [OCHK, 128, DCH, 128], BF, kind="ExternalInput")
    apekv_d = nc.dram_tensor("ape_kv", [4, 8, 128], BF, kind="ExternalInput")
    ind_d = nc.dram_tensor("ind", [4, 528], BF, kind="ExternalInput")
    cos_d = nc.dram_tensor("cos_t", [128, 512], F32, kind="ExternalInput")
    sin_d = nc.dram_tensor("sin_t", [128, 512], F32, kind="ExternalInput")
    h_d = nc.dram_tensor("hmat", [128, 4, 512], BF, kind="ExternalInput")
    zmask_d = nc.dram_tensor("zmask", [128, 1], F32, kind="ExternalInput")
    out_d = nc.dram_tensor("out", [4 * NBLK, 512], F32, kind="ExternalOutput")

    X = mybir.AxisListType.X

    def g4(tile_ap):
        return tile_ap.rearrange("p (b r) -> p b r", r=4)

    with TileContext(nc) as tc:
        with (
            tc.tile_pool(name="const", bufs=1) as constp,
            tc.tile_pool(name="xt", bufs=2) as xtp,
            tc.tile_pool(name="wt", bufs=7) as wtp,
            tc.tile_pool(name="sb", bufs=2) as sbp,
            tc.tile_pool(name="pl", bufs=2) as plp,
            tc.tile_pool(name="sm", bufs=2) as smp,
            tc.tile_pool(name="osb", bufs=2) as outp,
            tc.tile_pool(name="proj", bufs=6, space="PSUM") as projp,
            tc.tile_pool(name="had", bufs=1, space="PSUM") as hadp,
            tc.tile_pool(name="var", bufs=1, space="PSUM") as varp,
        ):
            # ---- weight prefetch for the first j-group goes out first ----
            w_tiles = {}
            for oc in (8, 12, 0, 4):
                w = wtp.tile([128, DCH, 128], BF, tag="w")
                w_tiles[oc] = w
            for k in range(0, DCH, 8):
                for oc in (8, 12, 0, 4):
                    nc.scalar.dma_start(out=w_tiles[oc][:, k:k + 8, :],
                                        in_=wp[oc][:, k:k + 8, :])

            # ---- constants: ape/ind/zmask first (needed by first groups) ----
            apekv_sb = constp.tile([4, 8, 128], BF, tag="apekv")
            nc.gpsimd.dma_start(out=apekv_sb[:], in_=apekv_d[:, :, :])
            ind_sb = constp.tile([4, 528], BF, tag="ind")
            nc.gpsimd.dma_start(out=ind_sb[:], in_=ind_d[:, :])
            zmask_sb = constp.tile([128, 1], F32, tag="zmask")
            nc.gpsimd.dma_start(out=zmask_sb[:], in_=zmask_d[:, :])
            cos_sb = constp.tile([128, 512], F32, tag="cos")
            sin_sb = constp.tile([128, 512], F32, tag="sin")
            h_sb = constp.tile([128, 4, 512], BF, tag="h")
            ones_sb = constp.tile([128, 1], BF, tag="ones")
            nc.vector.memset(ones_sb[:], 1.0)

            # PE p-state warmup: a few dummy matmuls while the first
            # weight/x DMAs are in flight, so real matmuls start at 2.4GHz
            warm_rhs = constp.tile([128, 512], BF, tag="warm")
            nc.gpsimd.memset(warm_rhs[:], 0.0)
            warm_ps = hadp.tile([128, 512], F32, tag="had", name="warm_ps")
            for wi in range(8):
                nc.tensor.matmul(warm_ps[:], lhsT=warm_rhs[:, 0:128],
                                 rhs=warm_rhs[:],
                                 start=(wi % 4 == 0), stop=(wi % 4 == 3))


            # deferred per-chunk state
            prev = None     # state dict of the previous m-chunk, for tail MMs
            for mch in range(MCH):
                r0 = MROWS * mch
                # ---- x^T tile: [128(d), 32 dchunk, 528 m] bf16 ----
                xt = xtp.tile([128, DCH, 528], BF, tag="xt")
                nbatch = 2 if mch == 0 else 4
                for c in range(0, DCH, nbatch):
                    nc.sync.dma_start(
                        out=xt[:, c:c + nbatch, :],
                        in_=xs[128 * c:128 * (c + nbatch), r0:r0 + 528]
                            .rearrange("(c p) m -> p c m", p=128),
                    )

                group = {}
                pooled = plp.tile([128, 4, NBLK], BF, tag="pooled")
                sq = plp.tile([128, 4, NBLK], BF, tag="sq")
                state = dict(pooled=pooled, sq=sq, mch=mch, emitted=0)

                def emit_stats_had(st, j):
                    """PE-side per-j tail: RMS-stats MM + Hadamard MM for
                    group j of chunk st (deferred until deps are ready)."""
                    p, s = st["pooled"], st["sq"]
                    nc.tensor.matmul(
                        st["var"][:], lhsT=s[:, j, :], rhs=ones_sb[:, 0:1],
                        start=(j == 0), stop=(j == 3), skip_group_check=True)
                    nc.tensor.matmul(
                        st["had"][:], lhsT=p[:, j, :], rhs=h_sb[:, j, :],
                        start=(j == 0), stop=(j == 3), skip_group_check=True)

                def emit_chunk_tail(st):
                    """rope + j=3 stats/had + scale + copy-out for chunk st."""
                    p = st["pooled"]
                    m = st["mch"]
                    # rope on group 3: cos=1/sin=0 on partitions <64, so
                    # only 64:128 need work; the pair swap is a DVE
                    # stream_shuffle (quadrant mask i^1), no PE/PSUM involved
                    cslice = cos_sb[64:128, m * NBLK:(m + 1) * NBLK]
                    sslice = sin_sb[64:128, m * NBLK:(m + 1) * NBLK]
                    sw = smp.tile([128, NBLK], BF, tag="sw")
                    nc.vector.stream_shuffle(sw[64:128, :], p[64:128, 3, :],
                                             [i ^ 1 for i in range(32)])
                    tmpc = smp.tile([128, NBLK], F32, tag="tmpc")
                    nc.vector.tensor_mul(tmpc[64:128, :], p[64:128, 3, :],
                                         cslice)
                    tmps = smp.tile([128, NBLK], F32, tag="tmps")
                    nc.vector.tensor_mul(tmps[64:128, :], sw[64:128, :],
                                         sslice)
                    nc.vector.tensor_add(p[64:128, 3, :], tmpc[64:128, :],
                                         tmps[64:128, :])
                    # j=3 squared stats on gpsimd (pre-rope value already in sq)
                    emit_stats_had(st, 3)
                    # scale = rsqrt(var/512 + eps) via DVE-only Newton
                    # iterations ([128,1] ops), so ACT keeps a single
                    # exp/copy table the whole kernel. Linear seed fitted
                    # for v in [0.25, 1.2]; two iterations -> <0.2% error.
                    v0 = smp.tile([128, 1], F32, tag="v0")
                    nc.vector.tensor_scalar(v0[:], st["var"][:],
                                            1.0 / 512.0, 1e-6,
                                            AluOpType.mult, AluOpType.add)
                    rsq = smp.tile([128, 1], F32, tag="rsq")
                    nc.vector.tensor_scalar(rsq[:], v0[:],
                                            -1.0424, 1.9774,
                                            AluOpType.mult, AluOpType.add)
                    tn = smp.tile([128, 1], F32, tag="tn")
                    for _ in range(2):
                        nc.vector.tensor_mul(tn[:], rsq[:], rsq[:])
                        nc.vector.tensor_mul(tn[:], tn[:], v0[:])
                        nc.vector.tensor_scalar(tn[:], tn[:], -0.5, 1.5,
                                                AluOpType.mult, AluOpType.add)
                        nc.vector.tensor_mul(rsq[:], rsq[:], tn[:])
                    out_sb = outp.tile([128, 512], F32, tag="out")
                    nc.scalar.activation(out_sb[:], st["had"][:],
                                         mybir.ActivationFunctionType.Copy,
                                         scale=rsq[:, 0:1])
                    nc.sync.dma_start(
                        out=out_d[m * NBLK:(m + 1) * NBLK, :], in_=out_sb[:])

                for j in range(4):
                    slots = (j + 8, j + 12, j, j + 4)
                    if mch == 0 and j == 0:
                        # d-outer for the very first group: each xt d-chunk is
                        # consumed by all 4 ocs right after its DMA lands, so
                        # the PE ramps at the HBM delivery rate instead of
                        # stalling a full xt pass per oc.
                        pss = []
                        for t, oc in enumerate(slots):
                            pss.append(projp.tile([128, MROWS], F32,
                                                  tag="proj", name=f"ps0_{t}"))
                        for d in range(DCH):
                            for t, oc in enumerate(slots):
                                off = 12 if oc in FIRST_HALF else 16
                                nc.tensor.matmul(
                                    pss[t][:],
                                    lhsT=w_tiles[oc][:, d, :],
                                    rhs=xt[:, d, off:off + MROWS],
                                    start=(d == 0),
                                    stop=(d == DCH - 1) and oc >= 8,
                                )
                        for t, oc in enumerate(slots):
                            wn = wtp.tile([128, DCH, 128], BF, tag="w")
                            nc.scalar.dma_start(out=wn[:], in_=wp[oc + 1])
                            w_tiles[oc + 1] = wn
                        for t, oc in enumerate(slots):
                            ps = pss[t]
                            off = 12 if oc in FIRST_HALF else 16
                            if oc < 8:
                                nc.tensor.matmul(
                                    ps[:], lhsT=apekv_sb[:, oc, :],
                                    rhs=ind_sb[:, off:off + MROWS],
                                    start=False, stop=True)
                                group[f"kvps{t - 2}"] = ps
                            else:
                                if t == 0:
                                    ec = sbp.tile([128, NBLK, 8], BF, tag="ec")
                                    group["ec"] = ec
                                ec = group["ec"]
                                nc.scalar.activation(
                                    ec[:, :, 4 * t:4 * t + 4], g4(ps[:]),
                                    mybir.ActivationFunctionType.Exp)
                                if oc < 12:
                                    nc.gpsimd.tensor_scalar_mul(
                                        ec[:, 0, 0:4], ec[:, 0, 0:4],
                                        zmask_sb[:, 0:1])
                    for t, oc in enumerate(slots):
                        if mch == 0 and j == 0:
                            break
                        # prefetch weights for the same t-slot of next group
                        if j < 3 or mch < MCH - 1:
                            noc = oc + 1 if j < 3 else oc - 3
                            wn = wtp.tile([128, DCH, 128], BF, tag="w")
                            nc.scalar.dma_start(out=wn[:], in_=wp[noc])
                        else:
                            wn = None
                        w = w_tiles[oc]
                        ps = projp.tile([128, MROWS], F32, tag="proj")
                        off = 12 if oc in FIRST_HALF else 16
                        is_kv = oc < 8
                        for d in range(DCH):
                            nc.tensor.matmul(
                                ps[:],
                                lhsT=w[:, d, :],
                                rhs=xt[:, d, off:off + MROWS],
                                start=(d == 0),
                                stop=(d == DCH - 1) and not is_kv,
                            )
                        if is_kv:
                            # +ape bias: K=4 matmul with one-hot phase rhs
                            nc.tensor.matmul(
                                ps[:],
                                lhsT=apekv_sb[:, oc, :],
                                rhs=ind_sb[:, off:off + MROWS],
                                start=False, stop=True,
                            )
                            group[f"kvps{t - 2}"] = ps
                        w_tiles[oc + 1 if j < 3 else oc - 3] = wn
                        # deferred PE tail work now that a proj group separates
                        # it from its producers
                        if t == 0 and j == 0 and prev is not None:
                            emit_chunk_tail(prev)
                            prev = None
                        if t == 2 and j > 0 and state["emitted"] < j:
                            emit_stats_had(state, j - 1)
                            state["emitted"] = j
                        if j == 1 and t == 0 and mch == 0:
                            # hmat feeds the first deferred had-MM (j1,t2)
                            nc.scalar.dma_start(out=h_sb[:], in_=h_d[:, :, :])
                        if j == 2 and t < 2 and mch == 0:
                            # rope constants: first consumed at mch1 j0;
                            # spread across j2's slots so no single weight
                            # prefetch is pushed far back
                            cd = ((cos_sb, cos_d[:, :]), (sin_sb, sin_d[:, :]))
                            nc.scalar.dma_start(out=cd[t][0][:], in_=cd[t][1])
                        if not is_kv:
                            # score chunk: e = exp(psum) into its half of the
                            # combined 8-slot window tile (bf16)
                            if t == 0:
                                ec = sbp.tile([128, NBLK, 8], BF, tag="ec")
                                group["ec"] = ec
                            ec = group["ec"]
                            nc.scalar.activation(
                                ec[:, :, 4 * t:4 * t + 4], g4(ps[:]),
                                mybir.ActivationFunctionType.Exp)
                            if mch == 0 and oc < 12:
                                nc.gpsimd.tensor_scalar_mul(
                                    ec[:, 0, 0:4], ec[:, 0, 0:4],
                                    zmask_sb[:, 0:1])

                    if j == 0:
                        state["var"] = varp.tile([128, 1], F32, tag="var", name="var_ps")
                        state["had"] = hadp.tile([128, 512], F32, tag="had", name="had_ps")

                    ec = group["ec"]
                    kvps1, kvps2 = group["kvps0"], group["kvps1"]
                    ssum = smp.tile([128, NBLK], F32, tag="ssum")
                    nc.vector.reduce_sum(ssum[:], ec[:], axis=X)
                    rinv = smp.tile([128, NBLK], F32, tag="rinv")
                    nc.vector.reciprocal(rinv[:], ssum[:])

                    # weighted-value chain on DVE, reading kv psum in place
                    pm = sbp.tile([128, NBLK, 8], BF, tag="pm")
                    nc.vector.tensor_mul(pm[:, :, 0:4], ec[:, :, 0:4],
                                         g4(kvps1[:]))
                    nc.vector.tensor_mul(pm[:, :, 4:8], ec[:, :, 4:8],
                                         g4(kvps2[:]))
                    qsum = smp.tile([128, NBLK], F32, tag="qsum")
                    nc.vector.reduce_sum(qsum[:], pm[:], axis=X)
                    nc.gpsimd.tensor_mul(pooled[:, j, :], qsum[:], rinv[:])
                    # squared pooled for RMS stats (pre-rope), on gpsimd
                    nc.gpsimd.tensor_mul(sq[:, j, :], pooled[:, j, :],
                                         pooled[:, j, :])

                prev = state
            emit_chunk_tail(prev)
    nc.compile()
    return nc


def _prep_shared(W_kv, W_gate, ape, norm_w, H):
    W = np.concatenate([W_kv, W_gate], axis=0).astype(np.float32)  # [2048, 4096]
    Wb = W.astype(BF16)
    wp = np.ascontiguousarray(
        Wb.T.reshape(DCH, 128, OCHK, 128).transpose(2, 1, 0, 3))  # [16,128,32,128]
    # ape bias as a K=4 matmul operand: ape_kv[r, oc, m] = ape[r, 128*oc+m]
    ape_kv = np.ascontiguousarray(
        ape.astype(np.float32)[:, :1024].reshape(4, 8, 128)).astype(BF16)
    ind = np.zeros((4, 528), np.float32)
    mm = np.arange(528)
    ind[(mm - 16) % 4, mm] = 1.0
    ind = ind.astype(BF16)
    hm = np.ascontiguousarray(
        (norm_w.astype(np.float32)[:, None] * H.astype(np.float32))
        .reshape(4, 128, 512).transpose(1, 0, 2)).astype(BF16)
    return wp, ape_kv, ind, hm


def _hadamard(n):
    h = np.array([[1.0]], dtype=np.float32)
    while h.shape[0] < n:
        h = np.block([[h, h], [h, -h]])
    return (h / np.sqrt(n)).astype(np.float32)


def _make_in_maps(x, W_kv, W_gate, ape, norm_w, freqs_cis):
    b, s, _ = x.shape
    H = _hadamard(512)
    wp, ape_kv, ind, hm = _prep_shared(W_kv, W_gate, ape, norm_w, H)

    # truncate-to-bf16 (hi-16 planes of the f32 words) and transpose once
    xh = x.reshape(b * s, DIM).view(BF16)[:, 1::2]
    xT = np.ascontiguousarray(xh.T)  # [4096, 16384]
    fr = freqs_cis[:, :, 0]  # [nb, 32]
    fi = freqs_cis[:, :, 1]

    in_maps = []
    for c in range(N_CORES):
        batch, half = c // 2, c % 2
        R0 = batch * s + half * ROWS
        xs = np.zeros((DIM, XS_ROWS), BF16)
        xs[:, 16:] = xT[:, R0:R0 + ROWS]
        if half == 1:
            xs[:, :16] = xT[:, R0 - 16:R0]

        g0 = half * 512
        bi = np.arange(g0, g0 + 512)
        cos_t = np.zeros((128, 512), np.float32)
        cos_t[:64] = 1.0
        cos_t[64:] = np.repeat(fr[bi].T, 2, axis=0)
        sin_t = np.zeros((128, 512), np.float32)
        st = np.repeat(fi[bi].T, 2, axis=0)
        st[0::2] *= -1.0
        sin_t[64:] = st

        zmask = np.full((128, 1), 0.0 if half == 0 else 1.0, np.float32)
        in_maps.append({
            "xs": xs, "wp": wp, "ape_kv": ape_kv, "ind": ind,
            "cos_t": cos_t, "sin_t": sin_t,
            "hmat": hm, "zmask": zmask,
        })
    return in_maps


def kernel(x, W_kv, W_gate, ape, norm_w, freqs_cis, start_pos=0):
    x = np.asarray(x, dtype=np.float32)
    W_kv = np.asarray(W_kv, dtype=np.float32)
    W_gate = np.asarray(W_gate, dtype=np.float32)
    ape = np.asarray(ape, dtype=np.float32)
    norm_w = np.asarray(norm_w, dtype=np.float32)
    freqs_cis = np.asarray(freqs_cis, dtype=np.float32)

    b, s, _ = x.shape
    nb = s // 4
    assert (b, s) == (4, 4096), (b, s)

    if "nc" not in _CACHE:
        _CACHE["nc"] = _build()
    nc = _CACHE["nc"]

    in_maps = _make_in_maps(x, W_kv, W_gate, ape, norm_w, freqs_cis)

    trace = os.environ.get("KERNEL_TRACE", "") not in ("", "0")
    res = run_bass_kernel_spmd(nc, in_maps, core_ids=list(range(N_CORES)),
                               trace=trace)
    kernel.last_results = res
    out = np.concatenate([res.results[c]["out"] for c in range(N_CORES)], axis=0)
    return np.ascontiguousarray(out.reshape(b, nb, 512))


# revision 41
# speedup vs baseline: 1.0027x; 1.0015x over previous
"""Trainium2 Bass kernel for nn_Compressor (sparse_attention block compressor).

Math (reference):
  proj = x @ [W_kv; W_gate]^T            # [b*s, 2048]
  kv   = proj[:, :1024] + ape[s%4]       # blockwise (RATIO=4) abs-pos bias
  sc   = proj[:, 1024:]
  window(blk) = {prev blk rows, ch 0:512} + {cur blk rows, ch 512:1024}
  pooled[blk, c] = softmax-gated channelwise pool over the 8-entry window
  out = (RMSNorm(pooled) -> rope on ch 448:512) @ H  (512x512 Hadamard)

Distribution: 8 cores, data-parallel over (batch, seq-half). Each core owns
2048 seq rows = 512 blocks; the 1-block halo is handled by shifting the
matmul rhs window by 4 rows (xs input carries 16 halo rows).

Schedule notes:
  * PE does 2048 projection matmuls (N=512, ~213ns each) plus small extras
    (ape-bias K=4 matmuls, bf16 Hadamard/RMS-stats); everything else is
    scheduled around keeping its issue rate at 1/213ns:
    - ape bias folded into the psum group as a K=4 matmul on a one-hot
      phase-indicator rhs, so kv psums need no drain: the pooling multiply
      reads them in place (score chunks run first in each group so kv psum
      banks get a long reuse window)
    - (mch0, j0) runs d-outer so the PE ramps at HBM delivery rate; weight
      DMAs stream from the ACT queue, big consts deferred past the head
    - per-j Hadamard/RMS-stats matmuls deferred one proj group so their
      waits resolve before they reach the PE queue head; whole chunk tails
      deferred into the next chunk's first proj group
    - a short dummy-matmul warmup raises the PE p-state during head DMAs
  * Pooling on split engines: exp windows land in one [128,b,8] bf16 tile
    (single reduce for the softmax denominator), products in one bf16 tile
    (single reduce for the numerator); final divide via DVE reciprocal,
    normalize multiply + squares on GpSimd. The rope pair-swap is a DVE
    stream_shuffle on partitions 64:128 (cos=1/sin=0 below), no PE/PSUM.
  * RMSNorm channel reduction via lhsT=squared-pooled, rhs=ones -> var lands
    as a [128,1] psum column (no transpose); rsqrt via DVE-only Newton
    iterations so ACT keeps a single exp/copy table the whole kernel.
  * Softmax without max-subtraction (scores ~N(0,1.3); fp32 exp cannot
    overflow; block-0 masking is a 0/1 multiply on exp, per-core mask).
"""

import os
import numpy as np
import ml_dtypes

import concourse.bass as bass
import concourse.bacc as bacc
import concourse.mybir as mybir
from concourse.alu_op_type import AluOpType
from concourse.tile import TileContext
from concourse.bass_utils import run_bass_kernel_spmd

BF16 = ml_dtypes.bfloat16
F32 = mybir.dt.float32
BF = mybir.dt.bfloat16

N_CORES = 8
DIM = 4096
OCH = 2048          # kv 1024 + gate 1024
ROWS = 2048         # own rows per core
XS_ROWS = 2064      # 16 halo/pad rows + 2048
MCH = 4             # m-chunks per core
MROWS = 512         # rows per m-chunk
NBLK = 128          # blocks per m-chunk
DCH = 32            # d chunks of 128
OCHK = 16           # o chunks of 128
# o-chunks 0..3 kv-first(prev), 4..7 kv-second(cur), 8..11 sc-first, 12..15 sc-second
FIRST_HALF = (0, 1, 2, 3, 8, 9, 10, 11)

_CACHE = {}


def _build():
    nc = bacc.Bacc("TRN2", target_bir_lowering=False, debug=False,
                   num_devices=N_CORES)
    xs = nc.dram_tensor("xs", [DIM, XS_ROWS], BF, kind="ExternalInput")
    wp = nc.dram_tensor("wp", [OCHK, 128, DCH, 128], BF, kind="ExternalInput")
    apekv_d = nc.dram_tensor("ape_kv", [4, 8, 128], BF, kind="ExternalInput")
    ind_d = nc.dram_tensor("ind", [4, 528], BF, kind="ExternalInput")
    cos_d = nc.dram_tensor("cos_t", [128, 512], F32, kind="ExternalInput")
    sin_d = nc.dram_tensor("sin_t", [128, 512], F32, kind="ExternalInput")
    h_d = nc.dram_tensor("hmat", [128, 4, 512], BF, kind="ExternalInput")
    zmask_d = nc.dram_tensor("zmask", [128, 1], F32, kind="ExternalInput")
    out_d = nc.dram_tensor("out", [4 * NBLK, 512], F32, kind="ExternalOutput")

    X = mybir.AxisListType.X

    def g4(tile_ap):
        return tile_ap.rearrange("p (b r) -> p b r", r=4)

    with TileContext(nc) as tc:
        with (
            tc.tile_pool(name="const", bufs=1) as constp,
            tc.tile_pool(name="xt", bufs=2) as xtp,
            tc.tile_pool(name="wt", bufs=7) as wtp,
            tc.tile_pool(name="sb", bufs=2) as sbp,
            tc.tile_pool(name="pl", bufs=2) as plp,
            tc.tile_pool(name="sm", bufs=2) as smp,
            tc.tile_pool(name="osb", bufs=2) as outp,
            tc.tile_pool(name="proj", bufs=6, space="PSUM") as projp,
            tc.tile_pool(name="had", bufs=1, space="PSUM") as hadp,
            tc.tile_pool(name="var", bufs=1, space="PSUM") as varp,
        ):
            # ---- weight prefetch for the first j-group goes out first ----
            w_tiles = {}
            for oc in (8, 12, 0, 4):
                w = wtp.tile([128, DCH, 128], BF, tag="w")
                w_tiles[oc] = w
            for k in range(0, DCH, 8):
                for oc in (8, 12, 0, 4):
                    nc.scalar.dma_start(out=w_tiles[oc][:, k:k + 8, :],
                                        in_=wp[oc][:, k:k + 8, :])

            # ---- constants: ape/ind/zmask first (needed by first groups) ----
            apekv_sb = constp.tile([4, 8, 128], BF, tag="apekv")
            nc.gpsimd.dma_start(out=apekv_sb[:], in_=apekv_d[:, :, :])
            ind_sb = constp.tile([4, 528], BF, tag="ind")
            nc.gpsimd.dma_start(out=ind_sb[:], in_=ind_d[:, :])
            zmask_sb = constp.tile([128, 1], F32, tag="zmask")
            nc.gpsimd.dma_start(out=zmask_sb[:], in_=zmask_d[:, :])
            cos_sb = constp.tile([128, 512], F32, tag="cos")
            sin_sb = constp.tile([128, 512], F32, tag="sin")
            h_sb = constp.tile([128, 4, 512], BF, tag="h")
            ones_sb = constp.tile([128, 1], BF, tag="ones")
            nc.vector.memset(ones_sb[:], 1.0)

            # PE p-state warmup: a few dummy matmuls while the first
            # weight/x DMAs are in flight, so real matmuls start at 2.4GHz
            warm_rhs = constp.tile([128, 512], BF, tag="warm")
            nc.gpsimd.memset(warm_rhs[:], 0.0)
            warm_ps = hadp.tile([128, 512], F32, tag="had", name="warm_ps")
            for wi in range(8):
                nc.tensor.matmul(warm_ps[:], lhsT=warm_rhs[:, 0:128],
                                 rhs=warm_rhs[:],
                                 start=(wi % 4 == 0), stop=(wi % 4 == 3))


            # deferred per-chunk state
            prev = None     # state dict of the previous m-chunk, for tail MMs
            for mch in range(MCH):
                r0 = MROWS * mch
                # ---- x^T tile: [128(d), 32 dchunk, 528 m] bf16 ----
                xt = xtp.tile([128, DCH, 528], BF, tag="xt")
                nbatch = 2 if mch == 0 else 4
                for c in range(0, DCH, nbatch):
                    nc.sync.dma_start(
                        out=xt[:, c:c + nbatch, :],
                        in_=xs[128 * c:128 * (c + nbatch), r0:r0 + 528]
                            .rearrange("(c p) m -> p c m", p=128),
                    )

                group = {}
                pooled = plp.tile([128, 4, NBLK], BF, tag="pooled")
                sq = plp.tile([128, 4, NBLK], BF, tag="sq")
                state = dict(pooled=pooled, sq=sq, mch=mch, emitted=0)

                def emit_stats_had(st, j):
                    """PE-side per-j tail: RMS-stats MM + Hadamard MM for
                    group j of chunk st (deferred until deps are ready)."""
                    p, s = st["pooled"], st["sq"]
                    nc.tensor.matmul(
                        st["var"][:], lhsT=s[:, j, :], rhs=ones_sb[:, 0:1],
                        start=(j == 0), stop=(j == 3), skip_group_check=True)
                    nc.tensor.matmul(
                        st["had"][:], lhsT=p[:, j, :], rhs=h_sb[:, j, :],
                        start=(j == 0), stop=(j == 3), skip_group_check=True)

                def emit_chunk_tail(st):
                    """rope + j=3 stats/had + scale + copy-out for chunk st."""
                    p = st["pooled"]
                    m = st["mch"]
                    # j=3 RMS stats first (pre-rope sq), so the scale
                    # pipeline starts while the rope runs on DVE
                    nc.tensor.matmul(
                        st["var"][:], lhsT=st["sq"][:, 3, :],
                        rhs=ones_sb[:, 0:1], start=False, stop=True,
                        skip_group_check=True)
                    # scale = rsqrt(var/512 + eps): psum read on DVE, then
                    # Newton iterations on GpSimd, concurrent with the rope.
                    # Linear seed fitted for v in [0.25, 1.2]; two
                    # iterations -> <0.2% error.
                    v0 = smp.tile([128, 1], F32, tag="v0")
                    nc.vector.tensor_scalar(v0[:], st["var"][:],
                                            1.0 / 512.0, 1e-6,
                                            AluOpType.mult, AluOpType.add)
                    rsq = smp.tile([128, 1], F32, tag="rsq")
                    nc.gpsimd.tensor_scalar(rsq[:], v0[:],
                                            -1.0424, 1.9774,
                                            AluOpType.mult, AluOpType.add)
                    tn = smp.tile([128, 1], F32, tag="tn")
                    for _ in range(2):
                        nc.gpsimd.tensor_mul(tn[:], rsq[:], rsq[:])
                        nc.gpsimd.tensor_mul(tn[:], tn[:], v0[:])
                        nc.gpsimd.tensor_scalar(tn[:], tn[:], -0.5, 1.5,
                                                AluOpType.mult, AluOpType.add)
                        nc.gpsimd.tensor_mul(rsq[:], rsq[:], tn[:])
                    # rope on group 3: cos=1/sin=0 on partitions <64, so
                    # only 64:128 need work; the pair swap is a DVE
                    # stream_shuffle (quadrant mask i^1), no PE/PSUM involved
                    cslice = cos_sb[64:128, m * NBLK:(m + 1) * NBLK]
                    sslice = sin_sb[64:128, m * NBLK:(m + 1) * NBLK]
                    sw = smp.tile([128, NBLK], BF, tag="sw")
                    nc.vector.stream_shuffle(sw[64:128, :], p[64:128, 3, :],
                                             [i ^ 1 for i in range(32)])
                    tmpc = smp.tile([128, NBLK], F32, tag="tmpc")
                    nc.vector.tensor_mul(tmpc[64:128, :], p[64:128, 3, :],
                                         cslice)
                    tmps = smp.tile([128, NBLK], F32, tag="tmps")
                    nc.vector.tensor_mul(tmps[64:128, :], sw[64:128, :],
                                         sslice)
                    nc.vector.tensor_add(p[64:128, 3, :], tmpc[64:128, :],
                                         tmps[64:128, :])
                    # Hadamard j=3 on the post-rope pooled
                    nc.tensor.matmul(
                        st["had"][:], lhsT=p[:, 3, :], rhs=h_sb[:, 3, :],
                        start=False, stop=True, skip_group_check=True)
                    out_sb = outp.tile([128, 512], F32, tag="out")
                    nc.scalar.activation(out_sb[:], st["had"][:],
                                         mybir.ActivationFunctionType.Copy,
                                         scale=rsq[:, 0:1])
                    nc.sync.dma_start(
                        out=out_d[m * NBLK:(m + 1) * NBLK, :], in_=out_sb[:])

                for j in range(4):
                    slots = (j + 8, j + 12, j, j + 4)
                    if mch == 0 and j == 0:
                        # d-outer for the very first group: each xt d-chunk is
                        # consumed by all 4 ocs right after its DMA lands, so
                        # the PE ramps at the HBM delivery rate instead of
                        # stalling a full xt pass per oc.
                        pss = []
                        for t, oc in enumerate(slots):
                            pss.append(projp.tile([128, MROWS], F32,
                                                  tag="proj", name=f"ps0_{t}"))
                        for d in range(DCH):
                            for t, oc in enumerate(slots):
                                off = 12 if oc in FIRST_HALF else 16
                                nc.tensor.matmul(
                                    pss[t][:],
                                    lhsT=w_tiles[oc][:, d, :],
                                    rhs=xt[:, d, off:off + MROWS],
                                    start=(d == 0),
                                    stop=(d == DCH - 1) and oc >= 8,
                                )
                        for t, oc in enumerate(slots):
                            wn = wtp.tile([128, DCH, 128], BF, tag="w")
                            nc.scalar.dma_start(out=wn[:], in_=wp[oc + 1])
                            w_tiles[oc + 1] = wn
                        for t, oc in enumerate(slots):
                            ps = pss[t]
                            off = 12 if oc in FIRST_HALF else 16
                            if oc < 8:
                                nc.tensor.matmul(
                                    ps[:], lhsT=apekv_sb[:, oc, :],
                                    rhs=ind_sb[:, off:off + MROWS],
                                    start=False, stop=True)
                                group[f"kvps{t - 2}"] = ps
                            else:
                                if t == 0:
                                    ec = sbp.tile([128, NBLK, 8], BF, tag="ec")
                                    group["ec"] = ec
                                ec = group["ec"]
                                nc.scalar.activation(
                                    ec[:, :, 4 * t:4 * t + 4], g4(ps[:]),
                                    mybir.ActivationFunctionType.Exp)
                                if oc < 12:
                                    nc.gpsimd.tensor_scalar_mul(
                                        ec[:, 0, 0:4], ec[:, 0, 0:4],
                                        zmask_sb[:, 0:1])
                    for t, oc in enumerate(slots):
                        if mch == 0 and j == 0:
                            break
                        # prefetch weights for the same t-slot of next group
                        if j < 3 or mch < MCH - 1:
                            noc = oc + 1 if j < 3 else oc - 3
                            wn = wtp.tile([128, DCH, 128], BF, tag="w")
                            nc.scalar.dma_start(out=wn[:], in_=wp[noc])
                        else:
                            wn = None
                        w = w_tiles[oc]
                        ps = projp.tile([128, MROWS], F32, tag="proj")
                        off = 12 if oc in FIRST_HALF else 16
                        is_kv = oc < 8
                        for d in range(DCH):
                            nc.tensor.matmul(
                                ps[:],
                                lhsT=w[:, d, :],
                                rhs=xt[:, d, off:off + MROWS],
                                start=(d == 0),
                                stop=(d == DCH - 1) and not is_kv,
                            )
                        if is_kv:
                            # +ape bias: K=4 matmul with one-hot phase rhs
                            nc.tensor.matmul(
                                ps[:],
                                lhsT=apekv_sb[:, oc, :],
                                rhs=ind_sb[:, off:off + MROWS],
                                start=False, stop=True,
                            )
                            group[f"kvps{t - 2}"] = ps
                        w_tiles[oc + 1 if j < 3 else oc - 3] = wn
                        # deferred PE tail work now that a proj group separates
                        # it from its producers
                        if t == 0 and j == 0 and prev is not None:
                            emit_chunk_tail(prev)
                            prev = None
                        if t == 2 and j > 0 and state["emitted"] < j:
                            emit_stats_had(state, j - 1)
                            state["emitted"] = j
                        if j == 1 and t == 0 and mch == 0:
                            # hmat feeds the first deferred had-MM (j1,t2)
                            nc.scalar.dma_start(out=h_sb[:], in_=h_d[:, :, :])
                        if j == 2 and t < 2 and mch == 0:
                            # rope constants: first consumed at mch1 j0;
                            # spread across j2's slots so no single weight
                            # prefetch is pushed far back
                            cd = ((cos_sb, cos_d[:, :]), (sin_sb, sin_d[:, :]))
                            nc.scalar.dma_start(out=cd[t][0][:], in_=cd[t][1])
                        if not is_kv:
                            # score chunk: e = exp(psum) into its half of the
                            # combined 8-slot window tile (bf16)
                            if t == 0:
                                ec = sbp.tile([128, NBLK, 8], BF, tag="ec")
                                group["ec"] = ec
                            ec = group["ec"]
                            nc.scalar.activation(
                                ec[:, :, 4 * t:4 * t + 4], g4(ps[:]),
                                mybir.ActivationFunctionType.Exp)
                            if mch == 0 and oc < 12:
                                nc.gpsimd.tensor_scalar_mul(
                                    ec[:, 0, 0:4], ec[:, 0, 0:4],
                                    zmask_sb[:, 0:1])

                    if j == 0:
                        state["var"] = varp.tile([128, 1], F32, tag="var", name="var_ps")
                        state["had"] = hadp.tile([128, 512], F32, tag="had", name="had_ps")

                    ec = group["ec"]
                    kvps1, kvps2 = group["kvps0"], group["kvps1"]
                    ssum = smp.tile([128, NBLK], F32, tag="ssum")
                    nc.vector.reduce_sum(ssum[:], ec[:], axis=X)
                    rinv = smp.tile([128, NBLK], F32, tag="rinv")
                    nc.vector.reciprocal(rinv[:], ssum[:])

                    # weighted-value chain on DVE, reading kv psum in place
                    pm = sbp.tile([128, NBLK, 8], BF, tag="pm")
                    nc.vector.tensor_mul(pm[:, :, 0:4], ec[:, :, 0:4],
                                         g4(kvps1[:]))
                    nc.vector.tensor_mul(pm[:, :, 4:8], ec[:, :, 4:8],
                                         g4(kvps2[:]))
                    qsum = smp.tile([128, NBLK], F32, tag="qsum")
                    nc.vector.reduce_sum(qsum[:], pm[:], axis=X)
                    nc.gpsimd.tensor_mul(pooled[:, j, :], qsum[:], rinv[:])
                    # squared pooled for RMS stats (pre-rope), on gpsimd
                    nc.gpsimd.tensor_mul(sq[:, j, :], pooled[:, j, :],
                                         pooled[:, j, :])

                prev = state
            emit_chunk_tail(prev)
    nc.compile()
    return nc


def _prep_shared(W_kv, W_gate, ape, norm_w, H):
    W = np.concatenate([W_kv, W_gate], axis=0).astype(np.float32)  # [2048, 4096]
    Wb = W.astype(BF16)
    wp = np.ascontiguousarray(
        Wb.T.reshape(DCH, 128, OCHK, 128).transpose(2, 1, 0, 3))  # [16,128,32,128]
    # ape bias as a K=4 matmul operand: ape_kv[r, oc, m] = ape[r, 128*oc+m]
    ape_kv = np.ascontiguousarray(
        ape.astype(np.float32)[:, :1024].reshape(4, 8, 128)).astype(BF16)
    ind = np.zeros((4, 528), np.float32)
    mm = np.arange(528)
    ind[(mm - 16) % 4, mm] = 1.0
    ind = ind.astype(BF16)
    hm = np.ascontiguousarray(
        (norm_w.astype(np.float32)[:, None] * H.astype(np.float32))
        .reshape(4, 128, 512).transpose(1, 0, 2)).astype(BF16)
    return wp, ape_kv, ind, hm


def _hadamard(n):
    h = np.array([[1.0]], dtype=np.float32)
    while h.shape[0] < n:
        h = np.block([[h, h], [h, -h]])
    return (h / np.sqrt(n)).astype(np.float32)


def _make_in_maps(x, W_kv, W_gate, ape, norm_w, freqs_cis):
    b, s, _ = x.shape
    H = _hadamard(512)
    wp, ape_kv, ind, hm = _prep_shared(W_kv, W_gate, ape, norm_w, H)

    # truncate-to-bf16 (hi-16 planes of the f32 words) and transpose once
    xh = x.reshape(b * s, DIM).view(BF16)[:, 1::2]
    xT = np.ascontiguousarray(xh.T)  # [4096, 16384]
    fr = freqs_cis[:, :, 0]  # [nb, 32]
    fi = freqs_cis[:, :, 1]

    in_maps = []
    for c in range(N_CORES):
        batch, half = c // 2, c % 2
        R0 = batch * s + half * ROWS
        xs = np.zeros((DIM, XS_ROWS), BF16)
        xs[:, 16:] = xT[:, R0:R0 + ROWS]
        if half == 1:
            xs[:, :16] = xT[:, R0 - 16:R0]

        g0 = half * 512
        bi = np.arange(g0, g0 + 512)
        cos_t = np.zeros((128, 512), np.float32)
        cos_t[:64] = 1.0
        cos_t[64:] = np.repeat(fr[bi].T, 2, axis=0)
        sin_t = np.zeros((128, 512), np.float32)
        st = np.repeat(fi[bi].T, 2, axis=0)
        st[0::2] *= -1.0
        sin_t[64:] = st

        zmask = np.full((128, 1), 0.0 if half == 0 else 1.0, np.float32)
        in_maps.append({
            "xs": xs, "wp": wp, "ape_kv": ape_kv, "ind": ind,
            "cos_t": cos_t, "sin_t": sin_t,
            "hmat": hm, "zmask": zmask,
        })
    return in_maps


def kernel(x, W_kv, W_gate, ape, norm_w, freqs_cis, start_pos=0):
    x = np.asarray(x, dtype=np.float32)
    W_kv = np.asarray(W_kv, dtype=np.float32)
    W_gate = np.asarray(W_gate, dtype=np.float32)
    ape = np.asarray(ape, dtype=np.float32)
    norm_w = np.asarray(norm_w, dtype=np.float32)
    freqs_cis = np.asarray(freqs_cis, dtype=np.float32)

    b, s, _ = x.shape
    nb = s // 4
    assert (b, s) == (4, 4096), (b, s)

    if "nc" not in _CACHE:
        _CACHE["nc"] = _build()
    nc = _CACHE["nc"]

    in_maps = _make_in_maps(x, W_kv, W_gate, ape, norm_w, freqs_cis)

    trace = os.environ.get("KERNEL_TRACE", "") not in ("", "0")
    res = run_bass_kernel_spmd(nc, in_maps, core_ids=list(range(N_CORES)),
                               trace=trace)
    kernel.last_results = res
    out = np.concatenate([res.results[c]["out"] for c in range(N_CORES)], axis=0)
    return np.ascontiguousarray(out.reshape(b, nb, 512))


# revision 42
# speedup vs baseline: 1.0084x; 1.0057x over previous
"""Trainium2 Bass kernel for nn_Compressor (sparse_attention block compressor).

Math (reference):
  proj = x @ [W_kv; W_gate]^T            # [b*s, 2048]
  kv   = proj[:, :1024] + ape[s%4]       # blockwise (RATIO=4) abs-pos bias
  sc   = proj[:, 1024:]
  window(blk) = {prev blk rows, ch 0:512} + {cur blk rows, ch 512:1024}
  pooled[blk, c] = softmax-gated channelwise pool over the 8-entry window
  out = (RMSNorm(pooled) -> rope on ch 448:512) @ H  (512x512 Hadamard)

Distribution: 8 cores, data-parallel over (batch, seq-half). Each core owns
2048 seq rows = 512 blocks; the 1-block halo is handled by shifting the
matmul rhs window by 4 rows (xs input carries 16 halo rows).

Schedule notes:
  * PE does 2048 projection matmuls (N=512, ~213ns each) plus small extras
    (ape-bias K=4 matmuls, bf16 Hadamard/RMS-stats); everything else is
    scheduled around keeping its issue rate at 1/213ns:
    - ape bias folded into the psum group as a K=4 matmul on a one-hot
      phase-indicator rhs, so kv psums need no drain: the pooling multiply
      reads them in place (score chunks run first in each group so kv psum
      banks get a long reuse window)
    - (mch0, j0) runs d-outer so the PE ramps at HBM delivery rate; weight
      DMAs stream from the ACT queue, big consts deferred past the head
    - per-j Hadamard/RMS-stats matmuls deferred one proj group so their
      waits resolve before they reach the PE queue head; whole chunk tails
      deferred into the next chunk's first proj group
    - a short dummy-matmul warmup raises the PE p-state during head DMAs
  * Pooling on split engines: exp windows land in one [128,b,8] bf16 tile
    (single reduce for the softmax denominator), products in one bf16 tile
    (single reduce for the numerator); final divide via DVE reciprocal,
    normalize multiply + squares on GpSimd. The rope pair-swap is a DVE
    stream_shuffle on partitions 64:128 (cos=1/sin=0 below), no PE/PSUM.
  * RMSNorm channel reduction via lhsT=squared-pooled, rhs=ones -> var lands
    as a [128,1] psum column (no transpose); rsqrt via DVE-only Newton
    iterations so ACT keeps a single exp/copy table the whole kernel.
  * Softmax without max-subtraction (scores ~N(0,1.3); fp32 exp cannot
    overflow; block-0 masking is a 0/1 multiply on exp, per-core mask).
"""

import os
import numpy as np
import ml_dtypes

import concourse.bass as bass
import concourse.bacc as bacc
import concourse.mybir as mybir
from concourse.alu_op_type import AluOpType
from concourse.tile import TileContext
from concourse.bass_utils import run_bass_kernel_spmd

BF16 = ml_dtypes.bfloat16
F32 = mybir.dt.float32
BF = mybir.dt.bfloat16

N_CORES = 8
DIM = 4096
OCH = 2048          # kv 1024 + gate 1024
ROWS = 2048         # own rows per core
XS_ROWS = 2064      # 16 halo/pad rows + 2048
MCH = 4             # m-chunks per core
MROWS = 512         # rows per m-chunk
NBLK = 128          # blocks per m-chunk
DCH = 32            # d chunks of 128
OCHK = 16           # o chunks of 128
# o-chunks 0..3 kv-first(prev), 4..7 kv-second(cur), 8..11 sc-first, 12..15 sc-second
FIRST_HALF = (0, 1, 2, 3, 8, 9, 10, 11)

_CACHE = {}


def _build():
    nc = bacc.Bacc("TRN2", target_bir_lowering=False, debug=False,
                   num_devices=N_CORES)
    xs = nc.dram_tensor("xs", [DIM, XS_ROWS], BF, kind="ExternalInput")
    wp = nc.dram_tensor("wp", [OCHK, 128, DCH, 128], BF, kind="ExternalInput")
    apekv_d = nc.dram_tensor("ape_kv", [4, 8, 128], BF, kind="ExternalInput")
    ind_d = nc.dram_tensor("ind", [4, 528], BF, kind="ExternalInput")
    cos_d = nc.dram_tensor("cos_t", [128, 512], F32, kind="ExternalInput")
    sin_d = nc.dram_tensor("sin_t", [128, 512], F32, kind="ExternalInput")
    h_d = nc.dram_tensor("hmat", [128, 4, 512], BF, kind="ExternalInput")
    zmask_d = nc.dram_tensor("zmask", [128, 1], F32, kind="ExternalInput")
    out_d = nc.dram_tensor("out", [4 * NBLK, 512], F32, kind="ExternalOutput")

    X = mybir.AxisListType.X

    def g4(tile_ap):
        return tile_ap.rearrange("p (b r) -> p b r", r=4)

    with TileContext(nc) as tc:
        with (
            tc.tile_pool(name="const", bufs=1) as constp,
            tc.tile_pool(name="xt", bufs=2) as xtp,
            tc.tile_pool(name="wt", bufs=7) as wtp,
            tc.tile_pool(name="sb", bufs=2) as sbp,
            tc.tile_pool(name="pl", bufs=2) as plp,
            tc.tile_pool(name="sm", bufs=2) as smp,
            tc.tile_pool(name="osb", bufs=2) as outp,
            tc.tile_pool(name="proj", bufs=6, space="PSUM") as projp,
            tc.tile_pool(name="had", bufs=1, space="PSUM") as hadp,
            tc.tile_pool(name="var", bufs=1, space="PSUM") as varp,
        ):
            # ---- weight prefetch for the first j-group goes out first ----
            w_tiles = {}
            for oc in (8, 12, 0, 4):
                w = wtp.tile([128, DCH, 128], BF, tag="w")
                w_tiles[oc] = w
            for k in range(0, DCH, 8):
                for oc in (8, 12, 0, 4):
                    nc.scalar.dma_start(out=w_tiles[oc][:, k:k + 8, :],
                                        in_=wp[oc][:, k:k + 8, :])

            # ---- constants: ape/ind/zmask first (needed by first groups) ----
            apekv_sb = constp.tile([4, 8, 128], BF, tag="apekv")
            nc.gpsimd.dma_start(out=apekv_sb[:], in_=apekv_d[:, :, :])
            ind_sb = constp.tile([4, 528], BF, tag="ind")
            nc.gpsimd.dma_start(out=ind_sb[:], in_=ind_d[:, :])
            zmask_sb = constp.tile([128, 1], F32, tag="zmask")
            nc.gpsimd.dma_start(out=zmask_sb[:], in_=zmask_d[:, :])
            cos_sb = constp.tile([128, 512], F32, tag="cos")
            sin_sb = constp.tile([128, 512], F32, tag="sin")
            h_sb = constp.tile([128, 4, 512], BF, tag="h")
            ones_sb = constp.tile([128, 1], BF, tag="ones")
            nc.vector.memset(ones_sb[:], 1.0)

            # PE p-state warmup: a few dummy matmuls while the first
            # weight/x DMAs are in flight, so real matmuls start at 2.4GHz
            warm_rhs = constp.tile([128, 512], BF, tag="warm")
            nc.gpsimd.memset(warm_rhs[:], 0.0)
            warm_ps = hadp.tile([128, 512], F32, tag="had", name="warm_ps")
            for wi in range(8):
                nc.tensor.matmul(warm_ps[:], lhsT=warm_rhs[:, 0:128],
                                 rhs=warm_rhs[:],
                                 start=(wi % 4 == 0), stop=(wi % 4 == 3))


            # deferred per-chunk state
            prev = None     # state dict of the previous m-chunk, for tail MMs
            for mch in range(MCH):
                r0 = MROWS * mch
                # ---- x^T tile: [128(d), 32 dchunk, 528 m] bf16 ----
                xt = xtp.tile([128, DCH, 528], BF, tag="xt")
                nbatch = 2 if mch == 0 else 4
                for c in range(0, DCH, nbatch):
                    nc.sync.dma_start(
                        out=xt[:, c:c + nbatch, :],
                        in_=xs[128 * c:128 * (c + nbatch), r0:r0 + 528]
                            .rearrange("(c p) m -> p c m", p=128),
                    )

                group = {}
                pooled = plp.tile([128, 4, NBLK], BF, tag="pooled")
                sq = plp.tile([128, 4, NBLK], BF, tag="sq")
                state = dict(pooled=pooled, sq=sq, mch=mch, emitted=0)

                def emit_stats_had(st, j):
                    """PE-side per-j tail: RMS-stats MM + Hadamard MM for
                    group j of chunk st (deferred until deps are ready)."""
                    p, s = st["pooled"], st["sq"]
                    nc.tensor.matmul(
                        st["var"][:], lhsT=s[:, j, :], rhs=ones_sb[:, 0:1],
                        start=(j == 0), stop=(j == 3), skip_group_check=True)
                    nc.tensor.matmul(
                        st["had"][:], lhsT=p[:, j, :], rhs=h_sb[:, j, :],
                        start=(j == 0), stop=(j == 3), skip_group_check=True)

                def emit_chunk_tail(st):
                    """rope + j=3 stats/had + scale + copy-out for chunk st."""
                    p = st["pooled"]
                    m = st["mch"]
                    # j=3 RMS stats first (pre-rope sq), so the scale
                    # pipeline starts while the rope runs on DVE
                    nc.tensor.matmul(
                        st["var"][:], lhsT=st["sq"][:, 3, :],
                        rhs=ones_sb[:, 0:1], start=False, stop=True,
                        skip_group_check=True)
                    # scale = rsqrt(var/512 + eps): psum read on DVE, then
                    # Newton iterations on GpSimd, concurrent with the rope.
                    # Linear seed fitted for v in [0.25, 1.2]; two
                    # iterations -> <0.2% error.
                    v0 = smp.tile([128, 1], F32, tag="v0")
                    nc.vector.tensor_scalar(v0[:], st["var"][:],
                                            1.0 / 512.0, 1e-6,
                                            AluOpType.mult, AluOpType.add)
                    rsq = smp.tile([128, 1], F32, tag="rsq")
                    nc.gpsimd.tensor_scalar(rsq[:], v0[:],
                                            -1.0424, 1.9774,
                                            AluOpType.mult, AluOpType.add)
                    tn = smp.tile([128, 1], F32, tag="tn")
                    for _ in range(2):
                        nc.gpsimd.tensor_mul(tn[:], rsq[:], rsq[:])
                        nc.gpsimd.tensor_mul(tn[:], tn[:], v0[:])
                        nc.gpsimd.tensor_scalar(tn[:], tn[:], -0.5, 1.5,
                                                AluOpType.mult, AluOpType.add)
                        nc.gpsimd.tensor_mul(rsq[:], rsq[:], tn[:])
                    # rope on group 3: cos=1/sin=0 on partitions <64, so
                    # only 64:128 need work; the pair swap is a DVE
                    # stream_shuffle (quadrant mask i^1), no PE/PSUM involved
                    cslice = cos_sb[64:128, m * NBLK:(m + 1) * NBLK]
                    sslice = sin_sb[64:128, m * NBLK:(m + 1) * NBLK]
                    sw = smp.tile([128, NBLK], BF, tag="sw")
                    nc.vector.stream_shuffle(sw[64:128, :], p[64:128, 3, :],
                                             [i ^ 1 for i in range(32)])
                    tmpc = smp.tile([128, NBLK], F32, tag="tmpc")
                    nc.vector.tensor_mul(tmpc[64:128, :], p[64:128, 3, :],
                                         cslice)
                    tmps = smp.tile([128, NBLK], F32, tag="tmps")
                    nc.vector.tensor_mul(tmps[64:128, :], sw[64:128, :],
                                         sslice)
                    nc.vector.tensor_add(p[64:128, 3, :], tmpc[64:128, :],
                                         tmps[64:128, :])
                    # Hadamard j=3 on the post-rope pooled
                    nc.tensor.matmul(
                        st["had"][:], lhsT=p[:, 3, :], rhs=h_sb[:, 3, :],
                        start=False, stop=True, skip_group_check=True)
                    out_sb = outp.tile([128, 512], F32, tag="out")
                    for h0 in (0, 256):
                        nc.scalar.activation(
                            out_sb[:, h0:h0 + 256],
                            st["had"][:, h0:h0 + 256],
                            mybir.ActivationFunctionType.Copy,
                            scale=rsq[:, 0:1])
                        nc.sync.dma_start(
                            out=out_d[m * NBLK:(m + 1) * NBLK, h0:h0 + 256],
                            in_=out_sb[:, h0:h0 + 256])

                for j in range(4):
                    slots = (j + 8, j + 12, j, j + 4)
                    if mch == 0 and j == 0:
                        # d-outer for the very first group: each xt d-chunk is
                        # consumed by all 4 ocs right after its DMA lands, so
                        # the PE ramps at the HBM delivery rate instead of
                        # stalling a full xt pass per oc.
                        pss = []
                        for t, oc in enumerate(slots):
                            pss.append(projp.tile([128, MROWS], F32,
                                                  tag="proj", name=f"ps0_{t}"))
                        for d in range(DCH):
                            for t, oc in enumerate(slots):
                                off = 12 if oc in FIRST_HALF else 16
                                nc.tensor.matmul(
                                    pss[t][:],
                                    lhsT=w_tiles[oc][:, d, :],
                                    rhs=xt[:, d, off:off + MROWS],
                                    start=(d == 0),
                                    stop=(d == DCH - 1) and oc >= 8,
                                )
                        for t, oc in enumerate(slots):
                            wn = wtp.tile([128, DCH, 128], BF, tag="w")
                            nc.scalar.dma_start(out=wn[:], in_=wp[oc + 1])
                            w_tiles[oc + 1] = wn
                        for t, oc in enumerate(slots):
                            ps = pss[t]
                            off = 12 if oc in FIRST_HALF else 16
                            if oc < 8:
                                nc.tensor.matmul(
                                    ps[:], lhsT=apekv_sb[:, oc, :],
                                    rhs=ind_sb[:, off:off + MROWS],
                                    start=False, stop=True)
                                group[f"kvps{t - 2}"] = ps
                            else:
                                if t == 0:
                                    ec = sbp.tile([128, NBLK, 8], BF, tag="ec")
                                    group["ec"] = ec
                                ec = group["ec"]
                                nc.scalar.activation(
                                    ec[:, :, 4 * t:4 * t + 4], g4(ps[:]),
                                    mybir.ActivationFunctionType.Exp)
                                if oc < 12:
                                    nc.gpsimd.tensor_scalar_mul(
                                        ec[:, 0, 0:4], ec[:, 0, 0:4],
                                        zmask_sb[:, 0:1])
                    for t, oc in enumerate(slots):
                        if mch == 0 and j == 0:
                            break
                        # prefetch weights for the same t-slot of next group
                        if j < 3 or mch < MCH - 1:
                            noc = oc + 1 if j < 3 else oc - 3
                            wn = wtp.tile([128, DCH, 128], BF, tag="w")
                            nc.scalar.dma_start(out=wn[:], in_=wp[noc])
                        else:
                            wn = None
                        w = w_tiles[oc]
                        ps = projp.tile([128, MROWS], F32, tag="proj")
                        off = 12 if oc in FIRST_HALF else 16
                        is_kv = oc < 8
                        for d in range(DCH):
                            nc.tensor.matmul(
                                ps[:],
                                lhsT=w[:, d, :],
                                rhs=xt[:, d, off:off + MROWS],
                                start=(d == 0),
                                stop=(d == DCH - 1) and not is_kv,
                            )
                        if is_kv:
                            # +ape bias: K=4 matmul with one-hot phase rhs
                            nc.tensor.matmul(
                                ps[:],
                                lhsT=apekv_sb[:, oc, :],
                                rhs=ind_sb[:, off:off + MROWS],
                                start=False, stop=True,
                            )
                            group[f"kvps{t - 2}"] = ps
                        w_tiles[oc + 1 if j < 3 else oc - 3] = wn
                        # deferred PE tail work now that a proj group separates
                        # it from its producers
                        if t == 0 and j == 0 and prev is not None:
                            emit_chunk_tail(prev)
                            prev = None
                        if t == 2 and j > 0 and state["emitted"] < j:
                            emit_stats_had(state, j - 1)
                            state["emitted"] = j
                        if j == 1 and t == 0 and mch == 0:
                            # hmat feeds the first deferred had-MM (j1,t2)
                            nc.scalar.dma_start(out=h_sb[:], in_=h_d[:, :, :])
                        if j == 2 and t < 2 and mch == 0:
                            # rope constants: first consumed at mch1 j0;
                            # spread across j2's slots so no single weight
                            # prefetch is pushed far back
                            cd = ((cos_sb, cos_d[:, :]), (sin_sb, sin_d[:, :]))
                            nc.scalar.dma_start(out=cd[t][0][:], in_=cd[t][1])
                        if not is_kv:
                            # score chunk: e = exp(psum) into its half of the
                            # combined 8-slot window tile (bf16)
                            if t == 0:
                                ec = sbp.tile([128, NBLK, 8], BF, tag="ec")
                                group["ec"] = ec
                            ec = group["ec"]
                            nc.scalar.activation(
                                ec[:, :, 4 * t:4 * t + 4], g4(ps[:]),
                                mybir.ActivationFunctionType.Exp)
                            if mch == 0 and oc < 12:
                                nc.gpsimd.tensor_scalar_mul(
                                    ec[:, 0, 0:4], ec[:, 0, 0:4],
                                    zmask_sb[:, 0:1])

                    if j == 0:
                        state["var"] = varp.tile([128, 1], F32, tag="var", name="var_ps")
                        state["had"] = hadp.tile([128, 512], F32, tag="had", name="had_ps")

                    ec = group["ec"]
                    kvps1, kvps2 = group["kvps0"], group["kvps1"]
                    ssum = smp.tile([128, NBLK], F32, tag="ssum")
                    nc.vector.reduce_sum(ssum[:], ec[:], axis=X)
                    rinv = smp.tile([128, NBLK], F32, tag="rinv")
                    nc.vector.reciprocal(rinv[:], ssum[:])

                    # weighted-value chain on DVE, reading kv psum in place
                    pm = sbp.tile([128, NBLK, 8], BF, tag="pm")
                    nc.vector.tensor_mul(pm[:, :, 0:4], ec[:, :, 0:4],
                                         g4(kvps1[:]))
                    nc.vector.tensor_mul(pm[:, :, 4:8], ec[:, :, 4:8],
                                         g4(kvps2[:]))
                    qsum = smp.tile([128, NBLK], F32, tag="qsum")
                    nc.vector.reduce_sum(qsum[:], pm[:], axis=X)
                    nc.gpsimd.tensor_mul(pooled[:, j, :], qsum[:], rinv[:])
                    # squared pooled for RMS stats (pre-rope), on gpsimd
                    nc.gpsimd.tensor_mul(sq[:, j, :], pooled[:, j, :],
                                         pooled[:, j, :])

                prev = state
            emit_chunk_tail(prev)
    nc.compile()
    return nc


def _prep_shared(W_kv, W_gate, ape, norm_w, H):
    W = np.concatenate([W_kv, W_gate], axis=0).astype(np.float32)  # [2048, 4096]
    Wb = W.astype(BF16)
    wp = np.ascontiguousarray(
        Wb.T.reshape(DCH, 128, OCHK, 128).transpose(2, 1, 0, 3))  # [16,128,32,128]
    # ape bias as a K=4 matmul operand: ape_kv[r, oc, m] = ape[r, 128*oc+m]
    ape_kv = np.ascontiguousarray(
        ape.astype(np.float32)[:, :1024].reshape(4, 8, 128)).astype(BF16)
    ind = np.zeros((4, 528), np.float32)
    mm = np.arange(528)
    ind[(mm - 16) % 4, mm] = 1.0
    ind = ind.astype(BF16)
    hm = np.ascontiguousarray(
        (norm_w.astype(np.float32)[:, None] * H.astype(np.float32))
        .reshape(4, 128, 512).transpose(1, 0, 2)).astype(BF16)
    return wp, ape_kv, ind, hm


def _hadamard(n):
    h = np.array([[1.0]], dtype=np.float32)
    while h.shape[0] < n:
        h = np.block([[h, h], [h, -h]])
    return (h / np.sqrt(n)).astype(np.float32)


def _make_in_maps(x, W_kv, W_gate, ape, norm_w, freqs_cis):
    b, s, _ = x.shape
    H = _hadamard(512)
    wp, ape_kv, ind, hm = _prep_shared(W_kv, W_gate, ape, norm_w, H)

    # truncate-to-bf16 (hi-16 planes of the f32 words) and transpose once
    xh = x.reshape(b * s, DIM).view(BF16)[:, 1::2]
    xT = np.ascontiguousarray(xh.T)  # [4096, 16384]
    fr = freqs_cis[:, :, 0]  # [nb, 32]
    fi = freqs_cis[:, :, 1]

    in_maps = []
    for c in range(N_CORES):
        batch, half = c // 2, c % 2
        R0 = batch * s + half * ROWS
        xs = np.zeros((DIM, XS_ROWS), BF16)
        xs[:, 16:] = xT[:, R0:R0 + ROWS]
        if half == 1:
            xs[:, :16] = xT[:, R0 - 16:R0]

        g0 = half * 512
        bi = np.arange(g0, g0 + 512)
        cos_t = np.zeros((128, 512), np.float32)
        cos_t[:64] = 1.0
        cos_t[64:] = np.repeat(fr[bi].T, 2, axis=0)
        sin_t = np.zeros((128, 512), np.float32)
        st = np.repeat(fi[bi].T, 2, axis=0)
        st[0::2] *= -1.0
        sin_t[64:] = st

        zmask = np.full((128, 1), 0.0 if half == 0 else 1.0, np.float32)
        in_maps.append({
            "xs": xs, "wp": wp, "ape_kv": ape_kv, "ind": ind,
            "cos_t": cos_t, "sin_t": sin_t,
            "hmat": hm, "zmask": zmask,
        })
    return in_maps


def kernel(x, W_kv, W_gate, ape, norm_w, freqs_cis, start_pos=0):
    x = np.asarray(x, dtype=np.float32)
    W_kv = np.asarray(W_kv, dtype=np.float32)
    W_gate = np.asarray(W_gate, dtype=np.float32)
    ape = np.asarray(ape, dtype=np.float32)
    norm_w = np.asarray(norm_w, dtype=np.float32)
    freqs_cis = np.asarray(freqs_cis, dtype=np.float32)

    b, s, _ = x.shape
    nb = s // 4
    assert (b, s) == (4, 4096), (b, s)

    if "nc" not in _CACHE:
        _CACHE["nc"] = _build()
    nc = _CACHE["nc"]

    in_maps = _make_in_maps(x, W_kv, W_gate, ape, norm_w, freqs_cis)

    trace = os.environ.get("KERNEL_TRACE", "") not in ("", "0")
    res = run_bass_kernel_spmd(nc, in_maps, core_ids=list(range(N_CORES)),
                               trace=trace)
    kernel.last_results = res
    out = np.concatenate([res.results[c]["out"] for c in range(N_CORES)], axis=0)
    return np.ascontiguousarray(out.reshape(b, nb, 512))
